# revision 1
# baseline (speedup 1.0000x reference)
"""DeepSigNet Trainium2 kernel (8 NeuronCores, data-parallel over batch).

Restructured depth-3 streamed path-signature + 1x1-conv network:
all sequential scans become free-dim cumsums (tensor_tensor_scan), all
signature/projection contractions become TensorE matmuls in float32r (tf32),
rank-1 outer-product structures are built via 0/1 selection matmuls + one
elementwise multiply.

Math (per batch element, channels C, increments dx_t = a_t - a_{t-1}, a_{-1}=0):
  s1_t = cumsum(dx),  s1p = exclusive prefix,  u = s1p + dx/2
  s2[i,j]_t = cumsum_t(u[i] dx[j])
  y3_t = cumsum_v g3_v with
  g3_v[h] = sum_k dx_v[k] * sum_{ij} W3[h,i,j,k] (s2p_v[i,j] + dx_v[i]dx_v[j]/6)
          + sum_i s1p_v[i] * sum_{jk} W3[h,i,j,k] dx_v[j]dx_v[k]/2
Projection of each signature level commutes with the cumsum, so nothing of
size C^3 is ever materialized; per-step work is dense matmuls over L=256.

Self-contained: hardcodes shapes from the problem spec
(x: (8, 256, 33) f32; W1 (8,40494); b1 (8,); W2 (4,8); b2 (4,); Wl (1,84); bl (1,)).
"""
from contextlib import ExitStack

import numpy as np

import concourse.bass as bass
import concourse.tile as tile
from concourse import mybir
from concourse.bass_utils import run_bass_kernel_spmd

F32 = mybir.dt.float32
F32R = mybir.dt.float32r
AO = mybir.AluOpType
AF = mybir.ActivationFunctionType

B, L, CIN = 8, 256, 33
C = CIN + 1          # 34
H = 8                # conv1 out channels
C2 = 4               # conv2 out channels = stage-2 path channels
NPAIR = C * C        # 1156
NSYM = C * (C + 1) // 2   # 595
NP2 = C2 * C2        # 16
NSYM2 = C2 * (C2 + 1) // 2  # 10
KH = H * C           # 272 (cols (h,k) / (h,i), h-major)

# ---------------------------------------------------------------------------
# Walrus in this environment rejects >1 sync wait/update per instruction;
# split extras onto NOP carriers (a preceding same-engine NOP wait is
# semantically identical).
MAX_WAITS = 1
MAX_UPD = 1


def _fix_multiwait(nc):
    for func in nc.m.functions:
        for block in func.blocks:
            new_insts = []
            for inst in block.instructions:
                si = inst.sync_info
                if si is not None and si.on_wait and len(si.on_wait) > MAX_WAITS:
                    waits = list(si.on_wait)
                    for w in waits[MAX_WAITS:]:
                        new_insts.append(mybir.InstNoOp(
                            name=nc.get_next_instruction_name(), ins=[], outs=[],
                            engine=inst.engine,
                            sync_info=mybir.SyncInfo(on_wait=[w], on_update=[])))
                    inst.sync_info = mybir.SyncInfo(
                        on_wait=waits[:MAX_WAITS],
                        on_update=list(si.on_update or []))
                new_insts.append(inst)
                si = inst.sync_info
                if si is not None and si.on_update and len(si.on_update) > MAX_UPD:
                    assert not type(inst).__name__.startswith("InstDMA")
                    upds = list(si.on_update)
                    inst.sync_info = mybir.SyncInfo(
                        on_wait=list(si.on_wait or []), on_update=upds[:MAX_UPD])
                    for u in upds[MAX_UPD:]:
                        new_insts.append(mybir.InstNoOp(
                            name=nc.get_next_instruction_name(), ins=[], outs=[],
                            engine=inst.engine,
                            sync_info=mybir.SyncInfo(on_wait=[], on_update=[u])))
            block.instructions[:] = new_insts
    return nc


# ---------------------------------------------------------------------------
# host-side constant prep

def _sym_pairs(c):
    return [(p, q) for p in range(c) for q in range(p, c)]


def _chunks(n, size=128):
    return [(s, min(s + size, n)) for s in range(0, n, size)]


def _cycled_runs(lo, hi, period):
    """Dest rows [lo,hi) with src row = r % period -> contiguous runs
    (dest_lo, dest_hi, src_lo, src_hi)."""
    runs = []
    r = lo
    while r < hi:
        i = r % period
        n = min(period - i, hi - r)
        runs.append((r, r + n, i, i + n))
        r += n
    return runs


def _block_runs(pairs, lo, hi):
    """Dest rows [lo,hi) of the sym-pair table with src row = pairs[r][1]."""
    runs = []
    r = lo
    while r < hi:
        p, q = pairs[r]
        n = 1
        while r + n < hi and pairs[r + n] == (p, q + n):
            n += 1
        runs.append((r, r + n, q, q + n))
        r += n
    return runs


def prep_consts(W1, b1, W2, b2, Wl, bl):
    W1 = np.asarray(W1, np.float32)
    Wl = np.asarray(Wl, np.float32)
    W11 = W1[:, :C].T.copy()                                    # (34, 8)
    W12 = W1[:, C:C + NPAIR].reshape(H, C, C)                   # [h, i, j]
    W3 = W1[:, C + NPAIR:].reshape(H, C, C, C)                  # [h, i, j, k]
    # s2 is computed scaled by 2 on device (uj' = 2u); halve its consumers
    W12p = W12.transpose(2, 1, 0).reshape(NPAIR, H).copy() / 2.0
    W3A2 = W3.transpose(2, 1, 0, 3).reshape(NPAIR, KH).copy() / 2.0
    pairs = _sym_pairs(C)
    W3hk = W3.transpose(1, 2, 0, 3).reshape(C, C, KH)           # [i, j, (h,k)]
    W3hi = W3.transpose(2, 3, 0, 1).reshape(C, C, KH)           # [j, k, (h,i)]
    W3S6 = np.zeros((NSYM, KH), np.float32)
    W3B2 = np.zeros((NSYM, KH), np.float32)
    for r, (p, q) in enumerate(pairs):
        if p == q:
            W3S6[r] = W3hk[p, p] / 6.0
            W3B2[r] = W3hi[p, p] / 2.0
        else:
            W3S6[r] = (W3hk[p, q] + W3hk[q, p]) / 6.0
            W3B2[r] = (W3hi[p, q] + W3hi[q, p]) / 2.0
    EJ = np.zeros((C, NPAIR), np.float32)
    for r in range(NPAIR):
        EJ[r // C, r] = 1.0
    EI = np.zeros((C, NSYM), np.float32)
    for r, (p, q) in enumerate(pairs):
        EI[p, r] = 1.0
    Ssel = np.zeros((KH, H), np.float32)
    for h in range(H):
        Ssel[h * C:(h + 1) * C, h] = 1.0

    Wl1 = Wl[:, :C2].T.copy()                                   # (4, 1)
    Wl2 = Wl[:, C2:C2 + NP2].reshape(C2, C2)                    # [i, j]
    Wl3 = Wl[:, C2 + NP2:].reshape(C2, C2, C2)                  # [i, j, k]
    Wl2p = Wl2.T.reshape(NP2, 1).copy() / 2.0
    Wl3A2 = Wl3.transpose(1, 0, 2).reshape(NP2, C2).copy() / 2.0
    pairs2 = _sym_pairs(C2)
    Wl3S6 = np.zeros((NSYM2, C2), np.float32)
    Wl3B2 = np.zeros((NSYM2, C2), np.float32)
    for r, (p, q) in enumerate(pairs2):
        if p == q:
            Wl3S6[r] = Wl3[p, p, :] / 6.0
            Wl3B2[r] = Wl3[:, p, p] / 2.0
        else:
            Wl3S6[r] = (Wl3[p, q, :] + Wl3[q, p, :]) / 6.0
            Wl3B2[r] = (Wl3[:, p, q] + Wl3[:, q, p]) / 2.0
    EJU = np.zeros((C, NPAIR), np.float32)
    for r in range(NPAIR):
        EJU[r % C, r] = 1.0
    EQ2 = np.zeros((C, NSYM), np.float32)
    for r, (p, q) in enumerate(pairs):
        EQ2[q, r] = 1.0
    EC3 = np.zeros((C, KH), np.float32)
    for r in range(KH):
        EC3[r % C, r] = 1.0
    EJc = np.zeros((C2, NP2), np.float32)
    for r in range(NP2):
        EJc[r // C2, r] = 1.0
    EIc = np.zeros((C2, NSYM2), np.float32)
    for r, (p, q) in enumerate(pairs2):
        EIc[p, r] = 1.0
    EJUc = np.zeros((C2, NP2), np.float32)
    for r in range(NP2):
        EJUc[r % C2, r] = 1.0
    EQ2c = np.zeros((C2, NSYM2), np.float32)
    for r, (p, q) in enumerate(pairs2):
        EQ2c[q, r] = 1.0

    return dict(
        EJU=EJU, EQ2=EQ2, EC3=EC3, EJUc=EJUc, EQ2c=EQ2c,
        W11=W11, W12p=W12p, W3A2=W3A2, W3S6=W3S6, W3B2=W3B2,
        EJ=EJ, EI=EI, Ssel=Ssel,
        W2T=np.asarray(W2, np.float32).T.copy(),
        b1=np.asarray(b1, np.float32).reshape(H, 1),
        b2c=np.asarray(b2, np.float32).reshape(C2, 1),
        bl=np.asarray(bl, np.float32).reshape(1, 1),
        Wl1=Wl1, Wl2p=Wl2p, Wl3A2=Wl3A2, Wl3S6=Wl3S6, Wl3B2=Wl3B2,
        EJc=EJc, EIc=EIc, onesc=np.ones((C2, 1), np.float32),
    )


TIME_ROW = np.linspace(0.0, 1.0, L, dtype=np.float32)[None, :]   # (1, 256)


# ---------------------------------------------------------------------------
# numpy mirror of the device dataflow (validation)

def np_forward(a_t, cst):
    pairs = _sym_pairs(C)
    inc = np.diff(a_t, axis=1, prepend=np.zeros((C, 1), np.float32))
    s1 = np.cumsum(inc, axis=1)
    s1p = np.concatenate([np.zeros((C, 1), np.float32), s1[:, :-1]], axis=1)
    u = inc * 0.5 + s1p
    dxj = cst["EJ"].T @ inc
    uj = u[np.arange(NPAIR) % C]
    pt = dxj * uj
    s2 = np.cumsum(pt, axis=1)
    s2p = np.concatenate([np.zeros((NPAIR, 1), np.float32), s2[:, :-1]], axis=1)
    dxi = cst["EI"].T @ inc
    dx2 = inc[[q for _, q in pairs]]
    b2t = dxi * dx2
    y12 = cst["W11"].T @ s1 + cst["W12p"].T @ s2
    M = cst["W3A2"].T @ s2p + cst["W3S6"].T @ b2t
    T = cst["W3B2"].T @ b2t
    dx3 = inc[np.arange(KH) % C]
    s1p3 = s1p[np.arange(KH) % C]
    g3 = cst["Ssel"].T @ (M * dx3) + cst["Ssel"].T @ (T * s1p3)
    y3 = np.cumsum(g3, axis=1)
    h = np.maximum(y12 + y3 + cst["b1"], 0.0)
    c = cst["W2T"].T @ h + cst["b2c"]
    pairs2 = _sym_pairs(C2)
    dc = np.diff(c, axis=1, prepend=np.zeros((C2, 1), np.float32))
    s1c = np.cumsum(dc, axis=1)
    s1cp = np.concatenate([np.zeros((C2, 1), np.float32), s1c[:, :-1]], axis=1)
    uc = dc * 0.5 + s1cp
    dcj = cst["EJc"].T @ dc
    uc4 = uc[np.arange(NP2) % C2]
    ptc = dcj * uc4
    s2c = np.cumsum(ptc, axis=1)
    s2cp = np.concatenate([np.zeros((NP2, 1), np.float32), s2c[:, :-1]], axis=1)
    dci = cst["EIc"].T @ dc
    dc2 = dc[[q for _, q in pairs2]]
    b2ct = dci * dc2
    yc = cst["Wl1"].T @ s1c + cst["Wl2p"].T @ s2c
    MC = cst["Wl3A2"].T @ s2cp + cst["Wl3S6"].T @ b2ct
    TC = cst["Wl3B2"].T @ b2ct
    g3c = cst["onesc"].T @ (MC * dc) + cst["onesc"].T @ (TC * s1cp)
    y3c = np.cumsum(g3c, axis=1)
    return yc + y3c + cst["bl"]


def np_kernel(x, W1, b1, W2, b2, Wl, bl):
    cst = prep_consts(W1, b1, W2, b2, Wl, bl)
    out = np.zeros((B, L, 1), np.float32)
    for b in range(B):
        a_t = np.concatenate([np.asarray(x[b], np.float32).T, TIME_ROW], 0)
        out[b, :, 0] = np_forward(a_t, cst)[0]
    return out




# ---------------------------------------------------------------------------
# constant packing: 3 DRAM tensors instead of ~55

CH_NP = _chunks(NPAIR)    # 10 chunks of <=128
CH_NS = _chunks(NSYM)     # 5 chunks
CH_KH = _chunks(KH)       # 3 chunks (128,128,16)


def pack_consts(cst):
    """Pack consts into cp34 (34, X) f32r, cp128 (128, Y) f32r, cpb (8,3) f32.
    Returns (arrays dict, offsets dict)."""
    off34 = {}
    cols34 = 0

    def reg34(name, arr):
        nonlocal cols34
        off34[name] = (arr.shape[0], arr.shape[1], cols34)
        cols34 += arr.shape[1]

    # stage-2 merged dc-selection: rows 0:16 dcj(EJc), 32:42 dcp(EIc),
    # 64:74 dcq(EQ2c)
    selc2 = np.zeros((C2, 74), np.float32)
    selc2[:, 0:NP2] = cst["EJc"]
    selc2[:, 32:32 + NSYM2] = cst["EIc"]
    selc2[:, 64:64 + NSYM2] = cst["EQ2c"]
    # stage-2 packed projection lhsTs (target rows 0:4 MC, 32:36 TC, 64 yc)
    l3_s2cp = np.zeros((NP2, 65), np.float32)
    l3_s2cp[:, 0:C2] = cst["Wl3A2"]
    l3_b2ct = np.zeros((NSYM2, 65), np.float32)
    l3_b2ct[:, 0:C2] = cst["Wl3S6"]
    l3_b2ct[:, 32:36] = cst["Wl3B2"]
    l3_s1c = np.zeros((C2, 65), np.float32)
    l3_s1c[:, 64] = cst["Wl1"][:, 0]
    l3_s2c = np.zeros((NP2, 65), np.float32)
    l3_s2c[:, 64] = cst["Wl2p"][:, 0]

    for name in ("EJ", "EJU", "EQ2", "EI", "EC3", "W11", "W2T",
                 "EJUc", "onesc"):
        reg34(name, cst[name])
    for name, arr in (("SELC2", selc2), ("L3S2CP", l3_s2cp),
                      ("L3B2CT", l3_b2ct), ("L3S1C", l3_s1c),
                      ("L3S2C", l3_s2c)):
        reg34(name, arr)
    cp34 = np.zeros((C, cols34), np.float32)
    for name, (rows, cols, off) in off34.items():
        arr = {"SELC2": selc2, "L3S2CP": l3_s2cp, "L3B2CT": l3_b2ct,
               "L3S1C": l3_s1c, "L3S2C": l3_s2c}.get(name)
        if arr is None:
            arr = cst[name]
        cp34[0:rows, off:off + cols] = arr

    off128 = {}
    cols128 = 0

    def reg128(name, nchunks, width):
        nonlocal cols128
        off128[name] = (cols128, width)
        cols128 += nchunks * width

    reg128("W3A2", len(CH_NP), KH)
    reg128("W3S6", len(CH_NS), KH)
    reg128("W3B2", len(CH_NS), KH)
    reg128("W12p", len(CH_NP), H)
    reg128("Ssel", len(CH_KH), H)
    cp128 = np.zeros((128, cols128), np.float32)
    for name, chunks in (("W3A2", CH_NP), ("W3S6", CH_NS), ("W3B2", CH_NS),
                         ("W12p", CH_NP), ("Ssel", CH_KH)):
        base, width = off128[name]
        for g, (lo, hi) in enumerate(chunks):
            cp128[0:hi - lo, base + g * width: base + (g + 1) * width] = \
                cst[name][lo:hi, :]

    cpb = np.zeros((H, 3), np.float32)
    cpb[0:H, 0] = cst["b1"][:, 0]
    cpb[0:C2, 1] = cst["b2c"][:, 0]
    cpb[0:1, 2] = cst["bl"][:, 0]
    return {"cp34": cp34, "cp128": cp128, "cpb": cpb}, (off34, off128)


# ---------------------------------------------------------------------------
# bass program


def build_nc(use_f32r=True, reps=1, stop_at=None):
    DT = F32R if use_f32r else F32
    nc = bass.Bass()

    # offsets must match pack_consts; compute them from a dummy pack
    dummy = prep_consts(np.zeros((H, C + NPAIR + C ** 3), np.float32),
                        np.zeros(H), np.zeros((C2, H)), np.zeros(C2),
                        np.zeros((1, C2 + NP2 + C2 ** 3)), np.zeros(1))
    packs, (off34, off128) = pack_consts(dummy)
    n34 = packs["cp34"].shape[1]
    n128 = packs["cp128"].shape[1]

    a_in = nc.dram_tensor("a_t", [C, L], F32, kind="ExternalInput")
    cp34_d = nc.dram_tensor("cp34", [C, n34], DT, kind="ExternalInput")
    cp128_d = nc.dram_tensor("cp128", [128, n128], DT, kind="ExternalInput")
    cpb_d = nc.dram_tensor("cpb", [H, 3], F32, kind="ExternalInput")
    out_d = nc.dram_tensor("out", [1, L], F32, kind="ExternalOutput")

    with tile.TileContext(nc) as tc, ExitStack() as ctx:
        wpool = ctx.enter_context(tc.tile_pool(name="weights", bufs=1))
        spool = ctx.enter_context(tc.tile_pool(name="state", bufs=1))
        # PSUM budget (8 banks): selps 2 + m 2 + t 2 + small 1 + g3 1 = 8
        pse = ctx.enter_context(tc.tile_pool(name="psel", bufs=1, space="PSUM"))
        psm = ctx.enter_context(tc.tile_pool(name="psm", bufs=2, space="PSUM"))
        pst = ctx.enter_context(tc.tile_pool(name="pst", bufs=2, space="PSUM"))
        psy = ctx.enter_context(tc.tile_pool(name="psy", bufs=1, space="PSUM"))
        psg = ctx.enter_context(tc.tile_pool(name="psg", bufs=1, space="PSUM"))

        def stile_(name, shape, dt, bufs=1):
            return spool.tile(list(shape), dt, name=name, tag=name,
                              bufs=max(bufs, 2 if reps > 1 else 1))

        # ---- const tiles (loaded once; DMAs emitted after first abuf) ----
        cp34_sb = wpool.tile([C, n34], DT, name="cp34", tag="cp34")
        cp128_sb = wpool.tile([128, n128], DT, name="cp128", tag="cp128")
        cpb_sb = wpool.tile([H, 3], F32, name="cpb", tag="cpb")
        zeros8 = wpool.tile([H, L], F32, name="zeros8", tag="zeros8")

        def c34(name, rows=None, cols=None, coloff=0):
            r, c_, off = off34[name]
            rows = r if rows is None else rows
            cols = c_ if cols is None else cols
            return cp34_sb[0:rows, off + coloff: off + coloff + cols]

        def c128(name, g, rows, mlo=0, mhi=None):
            base, width = off128[name]
            mhi = width if mhi is None else mhi
            return cp128_sb[0:rows, base + g * width + mlo:
                            base + g * width + mhi]

        def cbias(col, rows):
            return cpb_sb[0:rows, col:col + 1]

        first_rep_dmas = []

        for rep in range(reps):
            def stile(name, shape, dt, bufs=1):
                return stile_(name, shape, dt, bufs)

            # ---- path load & increments (first, so compute starts early) --
            abuf = stile("abuf", (C, L + 1), F32)
            nc.gpsimd.memset(abuf[:, 0:1], 0.0)
            nc.sync.dma_start(abuf[:, 1:L + 1], a_in[:])
            if rep == 0:
                # const DMAs immediately after the path DMA; they overlap
                # the early (weight-free) compute. cp128 split for
                # transfer parallelism.
                q34 = n34 // 4
                for i in range(4):
                    hi34 = n34 if i == 3 else (i + 1) * q34
                    nc.sync.dma_start(cp34_sb[:, i * q34:hi34],
                                      cp34_d[:, i * q34:hi34])
                q = n128 // 4
                for i in range(4):
                    hi2 = n128 if i == 3 else (i + 1) * q
                    nc.sync.dma_start(cp128_sb[:, i * q:hi2],
                                      cp128_d[:, i * q:hi2])
                nc.sync.dma_start(cpb_sb[:], cpb_d[:])
                nc.gpsimd.memset(zeros8[:], 0.0)
            inc = stile("inc", (C, L), DT)
            nc.vector.tensor_sub(inc[:], abuf[:, 1:L + 1], abuf[:, 0:L])

            s1buf = stile("s1buf", (C, L + 1), DT)
            nc.gpsimd.memset(s1buf[:, 0:1].bitcast(F32), 0.0)
            nc.vector.tensor_tensor_scan(
                s1buf[:, 1:L + 1], inc[:], inc[:], 0.0,
                op0=AO.add, op1=AO.bypass)
            s1p = s1buf[:, 0:L]
            s1incl = s1buf[:, 1:L + 1]
            if stop_at == "s1":
                dbg = stile("dbg", (1, L), F32)
                nc.vector.tensor_copy(dbg[:], s1buf[0:1, 1:L + 1])
                nc.sync.dma_start(out_d[:], dbg[:])
                continue

            # ---- P^T tiles and s2 scans -------------------------------
            # u = 0.5*inc + s1p is folded into the uj selection:
            # uj = EJUH @ inc + EJU @ s1p  (PSUM accumulation)
            s2bufs = []
            for g, (lo, hi) in enumerate(CH_NP):
                n = hi - lo
                sel_ps = pse.tile([128, 2 * L], F32, name="selps",
                                  tag="selps", bufs=2)
                nc.tensor.matmul(sel_ps[0:n, 0:L],
                                 c34("EJ", cols=n, coloff=lo),
                                 inc[:], start=True, stop=True)
                nc.tensor.matmul(sel_ps[0:n, L:2 * L],
                                 c34("EJU", cols=n, coloff=lo),
                                 s1p, start=True, stop=False)
                nc.tensor.matmul(sel_ps[0:n, L:2 * L],
                                 c34("EJU", cols=n, coloff=lo),
                                 s1incl, start=False, stop=True)
                uj = stile(f"uj{g}", (n, L), DT)
                nc.scalar.copy(uj[:], sel_ps[0:n, L:2 * L])
                pt = stile(f"pt{g}", (n, L), DT)
                nc.vector.tensor_mul(pt[:], sel_ps[0:n, 0:L], uj[:])
                s2b = stile(f"s2buf{g}", (n, L + 1), DT)
                nc.gpsimd.memset(s2b[:, 0:1].bitcast(F32), 0.0)
                nc.vector.tensor_tensor_scan(
                    s2b[:, 1:L + 1], pt[:], pt[:], 0.0,
                    op0=AO.add, op1=AO.bypass)
                s2bufs.append(s2b)

            # ---- B2T tiles -------------------------------------------
            b2ts = []
            for g, (lo, hi) in enumerate(CH_NS):
                n = hi - lo
                sel_ps = pse.tile([128, 2 * L], F32, name="selps",
                                  tag="selps", bufs=2)
                nc.tensor.matmul(sel_ps[0:n, 0:L],
                                 c34("EI", cols=n, coloff=lo),
                                 inc[:], start=True, stop=True)
                nc.tensor.matmul(sel_ps[0:n, L:2 * L],
                                 c34("EQ2", cols=n, coloff=lo), inc[:],
                                 start=True, stop=True)
                dx2 = stile(f"dx2{g}", (n, L), DT)
                nc.scalar.copy(dx2[:], sel_ps[0:n, L:2 * L])
                b2t = stile(f"b2t{g}", (n, L), DT)
                nc.vector.tensor_mul(b2t[:], sel_ps[0:n, 0:L], dx2[:])
                b2ts.append(b2t)

            # ---- y12 = W11.s1 + W12p.s2 ------------------------------
            y12_ps = psy.tile([H, L], F32, name="ps_y12", tag="ps_y12")
            nc.tensor.matmul(y12_ps[:], c34("W11"), s1incl,
                             start=True, stop=False)
            for g, (lo, hi) in enumerate(CH_NP):
                n = hi - lo
                nc.tensor.matmul(y12_ps[:], c128("W12p", g, n),
                                 s2bufs[g][0:n, 1:L + 1],
                                 start=False, stop=(g == len(CH_NP) - 1))
            if stop_at == "y12":
                dbg = stile("dbg", (1, L), F32)
                nc.vector.tensor_copy(dbg[:], y12_ps[0:1, :])
                nc.sync.dma_start(out_d[:], dbg[:])
                continue

            # ---- per column-tile: M, T, Z, g3 accumulation -----------
            g3_ps = psg.tile([H, L], F32, name="ps_g3", tag="ps_g3")
            for mt, (clo, chi) in enumerate(CH_KH):
                mn = chi - clo
                mp = psm.tile([mn, L], F32, name="ps_m", tag="ps_m", bufs=2)
                for g, (lo, hi) in enumerate(CH_NP):
                    n = hi - lo
                    nc.tensor.matmul(mp[:], c128("W3A2", g, n, clo, chi),
                                     s2bufs[g][0:n, 0:L],
                                     start=(g == 0), stop=False)
                for g, (lo, hi) in enumerate(CH_NS):
                    n = hi - lo
                    nc.tensor.matmul(mp[:], c128("W3S6", g, n, clo, chi),
                                     b2ts[g][:],
                                     start=False, stop=(g == len(CH_NS) - 1))
                c3_ps = pse.tile([128, 2 * L], F32, name="selps",
                                 tag="selps", bufs=2)
                nc.tensor.matmul(c3_ps[0:mn, 0:L],
                                 c34("EC3", cols=mn, coloff=clo), inc[:],
                                 start=True, stop=True)
                nc.tensor.matmul(c3_ps[0:mn, L:2 * L],
                                 c34("EC3", cols=mn, coloff=clo), s1p,
                                 start=True, stop=True)
                dx3 = stile(f"dx3_{mt}", (mn, L), DT)
                nc.scalar.copy(dx3[:], c3_ps[0:mn, 0:L])
                za = stile(f"za{mt}", (mn, L), DT)
                nc.vector.tensor_mul(za[:], mp[:], dx3[:])
                nc.tensor.matmul(g3_ps[:], c128("Ssel", mt, mn), za[:],
                                 start=(mt == 0), stop=False)

                tp = pst.tile([mn, L], F32, name="ps_t", tag="ps_t", bufs=2)
                for g, (lo, hi) in enumerate(CH_NS):
                    n = hi - lo
                    nc.tensor.matmul(tp[:], c128("W3B2", g, n, clo, chi),
                                     b2ts[g][:],
                                     start=(g == 0), stop=(g == len(CH_NS) - 1))
                s1p3 = stile(f"s1p3_{mt}", (mn, L), DT)
                nc.scalar.copy(s1p3[:], c3_ps[0:mn, L:2 * L])
                zb = stile(f"zb{mt}", (mn, L), DT)
                nc.vector.tensor_mul(zb[:], tp[:], s1p3[:])
                nc.tensor.matmul(g3_ps[:], c128("Ssel", mt, mn), zb[:],
                                 start=False, stop=(mt == len(CH_KH) - 1))

            y3_sb = stile("y3", (H, L), F32)
            nc.vector.tensor_tensor_scan(
                y3_sb[:], g3_ps[:], zeros8[:], 0.0,
                op0=AO.add, op1=AO.bypass)

            # ---- h = relu(y12 + y3 + b1); c = W2 h + b2 ---------------
            ypre = stile("ypre", (H, L), F32)
            nc.vector.tensor_add(ypre[:], y12_ps[:], y3_sb[:])
            hrelu = stile("hrelu", (H, L), DT)
            nc.scalar.activation(hrelu[:], ypre[:], AF.Relu,
                                 bias=cbias(0, H))
            c_ps = psy.tile([C2, L], F32, name="ps_y12", tag="ps_y12")
            nc.tensor.matmul(c_ps[:], c34("W2T"), hrelu[:],
                             start=True, stop=True)
            cbuf = stile("cbuf", (C2, L + 1), F32)
            nc.gpsimd.memset(cbuf[:, 0:1], 0.0)
            nc.scalar.activation(cbuf[:, 1:L + 1], c_ps[:], AF.Identity,
                                 bias=cbias(1, C2))
            if stop_at == "c":
                nc.sync.dma_start(out_d[:], cbuf[0:1, 1:L + 1])
                continue

            # ---- stage 2 ----------------------------------------------
            dc = stile("dc", (C2, L), DT)
            nc.vector.tensor_sub(dc[:], cbuf[:, 1:L + 1], cbuf[:, 0:L])
            s1cbuf = stile("s1cbuf", (C2, L + 1), DT)
            nc.gpsimd.memset(s1cbuf[:, 0:1].bitcast(F32), 0.0)
            nc.vector.tensor_tensor_scan(
                s1cbuf[:, 1:L + 1], dc[:], dc[:], 0.0,
                op0=AO.add, op1=AO.bypass)
            s1cp = s1cbuf[:, 0:L]

            # merged dc-selections: rows 0:16 dcj, 32:42 dc[p], 64:74 dc[q];
            # second half: uc4 = EJUHc @ dc + EJUc @ s1cp
            selc_ps = pse.tile([128, 2 * L], F32, name="selps",
                               tag="selps", bufs=2)
            nc.tensor.matmul(selc_ps[0:74, 0:L], c34("SELC2"), dc[:],
                             start=True, stop=True)
            nc.tensor.matmul(selc_ps[0:NP2, L:2 * L], c34("EJUc"), s1cp,
                             start=True, stop=False)
            nc.tensor.matmul(selc_ps[0:NP2, L:2 * L], c34("EJUc"),
                             s1cbuf[:, 1:L + 1], start=False, stop=True)
            uc4 = stile("uc4", (NP2, L), DT)
            nc.scalar.copy(uc4[:], selc_ps[0:NP2, L:2 * L])
            ptc = stile("ptc", (NP2, L), DT)
            nc.vector.tensor_mul(ptc[:], selc_ps[0:NP2, 0:L], uc4[:])
            s2cbuf = stile("s2cbuf", (NP2, L + 1), DT)
            nc.gpsimd.memset(s2cbuf[:, 0:1].bitcast(F32), 0.0)
            nc.vector.tensor_tensor_scan(
                s2cbuf[:, 1:L + 1], ptc[:], ptc[:], 0.0,
                op0=AO.add, op1=AO.bypass)
            dcq = stile("dcq", (NSYM2, L), DT)
            nc.scalar.copy(dcq[:], selc_ps[64:64 + NSYM2, 0:L])
            b2ct = stile("b2ct", (NSYM2, L), DT)
            nc.vector.tensor_mul(b2ct[:], selc_ps[32:32 + NSYM2, 0:L], dcq[:])

            # packed projections: rows 0:4 MC, 32:36 TC, 64 yc
            proj_ps = psy.tile([65, L], F32, name="ps_y12", tag="ps_y12")
            nc.tensor.matmul(proj_ps[:], c34("L3S2CP", cols=65),
                             s2cbuf[:, 0:L], start=True, stop=False)
            nc.tensor.matmul(proj_ps[:], c34("L3B2CT", cols=65), b2ct[:],
                             start=False, stop=False)
            nc.tensor.matmul(proj_ps[:], c34("L3S1C", cols=65),
                             s1cbuf[:, 1:L + 1], start=False, stop=False)
            nc.tensor.matmul(proj_ps[:], c34("L3S2C", cols=65),
                             s2cbuf[:, 1:L + 1], start=False, stop=True)

            zac = stile("zac", (C2, L), DT)
            nc.vector.tensor_mul(zac[:], proj_ps[0:C2, :], dc[:])
            zbc = stile("zbc", (C2, L), DT)
            nc.vector.tensor_mul(zbc[:], proj_ps[32:36, :], s1cp)
            g3c_ps = psg.tile([1, L], F32, name="ps_g3", tag="ps_g3")
            nc.tensor.matmul(g3c_ps[:], c34("onesc"), zac[:],
                             start=True, stop=False)
            nc.tensor.matmul(g3c_ps[:], c34("onesc"), zbc[:],
                             start=False, stop=True)
            y3c = stile("y3c", (1, L), F32)
            nc.vector.tensor_tensor_scan(
                y3c[:], g3c_ps[:], zeros8[0:1, :], 0.0,
                op0=AO.add, op1=AO.bypass)
            osum = stile("osum", (1, L), F32)
            nc.vector.tensor_add(osum[:], y3c[:], proj_ps[64:65, :])
            out_sb = stile("out_sb", (1, L), F32)
            nc.scalar.activation(out_sb[:], osum[:], AF.Identity,
                                 bias=cbias(2, 1))
            nc.sync.dma_start(out_d[:], out_sb[:])

    _fix_multiwait(nc)
    return nc

# ---------------------------------------------------------------------------
_CACHE = {}


def _get_nc(use_f32r=True, reps=1, stop_at=None):
    key = (use_f32r, reps, stop_at)
    if key not in _CACHE:
        _CACHE[key] = build_nc(use_f32r=use_f32r, reps=reps, stop_at=stop_at)
    return _CACHE[key]


def make_in_maps(x, W1, b1, W2, b2, Wl, bl):
    cst = prep_consts(W1, b1, W2, b2, Wl, bl)
    packs, _ = pack_consts(cst)
    consts = {k: np.ascontiguousarray(v, np.float32)
              for k, v in packs.items()}
    in_maps = []
    for b in range(B):
        a_t = np.concatenate(
            [np.asarray(x[b], np.float32).T, TIME_ROW], axis=0)
        m = dict(consts)
        m["a_t"] = np.ascontiguousarray(a_t)
        in_maps.append(m)
    return in_maps, cst


def run(x, W1, b1, W2, b2, Wl, bl, use_f32r=True, reps=1, stop_at=None, **kwargs):
    nc = _get_nc(use_f32r=use_f32r, reps=reps, stop_at=stop_at)
    in_maps, _ = make_in_maps(x, W1, b1, W2, b2, Wl, bl)
    return run_bass_kernel_spmd(nc, in_maps, core_ids=list(range(B)), **kwargs)


def kernel(x, W1, b1, W2, b2, Wl, bl):
    res = run(x, W1, b1, W2, b2, Wl, bl)
    out = np.stack([res.results[b]["out"].reshape(L, 1) for b in range(B)])
    return out.astype(np.float32)

